# revision 24
# baseline (speedup 1.0000x reference)
"""GNN attention kernel for Trainium2, SPMD over 8 NeuronCores.

Reference computation (per batch b, head h):
    Xp   = X @ Wp[h] + bp[h]                  [N, DH]
    Xc   = Xp @ C[h].T                        [N, DH]
    S    = Xc @ Xp.T                          [N, N]
    attn = tanh(A * S) = A * tanh(S)          (A is binary, tanh(0)=0)
    Xh   = attn @ Xp                          [N, DH]
    out  = relu(concat_h Xh)                  [N, DOUT]

Sharding: data-parallel over batch B=32 -> 4 batches per core. No collectives.

Per-core dataflow (bf16 on PE, f32 PSUM accumulate):
  - X[b], A[b] DMA'd natural f32. Their bf16 forms are obtained by viewing
    the f32 tiles as uint16 pairs and feeding only the high half-words
    (= truncated bf16) to the PE transposes: halves transpose cycles and
    the PSUM drains run in DVE 2x mode (bf16 in PSUM). A is 0/1 so the
    truncation is exact.
  - Xp  [m, (h k)] = XT.T @ Wp  (+bias on the drain when present)
  - XpT [(2h k), q, n]: PE transpose of Xp tiles (bias included for free).
  - XcT via pair-packed block-diagonal Cpad2: one K=128 matmul per
    (pair, nh) yields both heads' XcT stacked on partitions; the drain
    scatters the two 64-row halves into a persistent zero-padded
    [128, H, N] tile (zero halves never rewritten) so the score matmuls
    can contract K=128 with cross-head terms hitting zeros.
  - Scores T_h[m, n] = xpt-block.T @ xct[h]; tanh on ACT (PSUM f32 ->
    SBUF bf16); mask-mul with AT on DVE only, in [128, 4096] chunks.
    (GPSIMD masking was removed: concurrent GPSIMD+DVE SBUF streaming
    inflated both engines ~3.4x via port contention.)
  - Aggregate: lhsT = attnT tile, rhs = Xp -> Xh[n, k] in PSUM, ReLU to
    f32 out tile on DVE, DMA out per (pair, j).
  - ACT runs tanh plus a slice of proj-stream Copy drains (same act
    table, fills its projection-phase gaps without gating the tanh
    stream); everything PSUM-draining stays off GPSIMD (no PSUM access).

Scheduling: engine queues execute in emission order and the scores/tanh
stream is ACT-gated (~1.1us/chunk vs ~0.45us of PE work), so emission
interleaves three streams at tanh-slot granularity: scores chunks of
batch b, aggregate half-chunks of the previous pair, and projection
pieces of batch b+1. Batch 0's projection additionally spreads its PSUM
tiles over the idle psT banks to deepen the fill pipeline.
"""

import os
import sys
import types
import numpy as np

import concourse.bass as bass
import concourse.tile as tile
from concourse import bacc, mybir
from concourse.bass_utils import run_bass_kernel_spmd
from concourse.masks import make_identity


def _install_ntff_hook():
    """The image's ``antenv`` lacks ``axon_hooks``; shim it so
    ``run_bass_kernel_spmd(trace=True)`` can capture NTFF profiles through
    the ctypes hook from ``trn_agent_boot``. Degrades silently."""
    if "antenv.axon_hooks" in sys.modules:
        return
    try:
        import antenv  # noqa: F401

        mod = types.ModuleType("antenv.axon_hooks")
        mod._hook = None

        def set_axon_ntff_profile_hook(h):
            mod._hook = h

        def get_axon_ntff_profile_hook():
            return mod._hook

        mod.set_axon_ntff_profile_hook = set_axon_ntff_profile_hook
        mod.get_axon_ntff_profile_hook = get_axon_ntff_profile_hook
        sys.modules["antenv.axon_hooks"] = mod
        from trn_agent_boot.trn_boot import _ntff_profile_via_ctypes

        hook = _ntff_profile_via_ctypes("/opt/axon/libaxon_pjrt.so")
        if hook is not None:
            mod._hook = hook
    except Exception:
        pass


_install_ntff_hook()

B, N, DIN, DOUT, H, DH = 32, 1024, 512, 512, 8, 64
NCORES = 8
BS = B // NCORES          # 4 batches per core
NCH = N // 128            # 8 n/m chunks of 128
DT = DIN // 128           # 4 d tiles
PAIRS = H // 2            # 4 head pairs

F32 = mybir.dt.float32
BF16 = mybir.dt.bfloat16
U16 = mybir.dt.uint16
AF = mybir.ActivationFunctionType

LAST_EXEC_NS = None
LAST_TRACE_DIR = None


def _hi_bf16(ap_f32):
    """View the high half-words of an f32 AP as (truncated) bf16."""
    u = ap_f32.bitcast(U16)
    ndim = len(u.shape)
    dims = " ".join(f"d{i}" for i in range(ndim - 1))
    v = u.rearrange(f"{dims} (n two) -> {dims} n two", two=2)
    v = v[tuple([slice(None)] * ndim + [1])]
    return v.bitcast(BF16)


def _build(with_bias: bool, n_batch: int = BS):
    nc = bacc.Bacc("TRN2", target_bir_lowering=False, debug=False,
                   num_devices=NCORES)
    X = nc.dram_tensor("X", [BS, N, DIN], F32, kind="ExternalInput").ap()
    A = nc.dram_tensor("A", [BS, N, N], F32, kind="ExternalInput").ap()
    Wp = nc.dram_tensor("Wp", [H, DIN, DH], F32, kind="ExternalInput").ap()
    C = nc.dram_tensor("C", [H, DH, DH], F32, kind="ExternalInput").ap()
    bp = None
    if with_bias:
        bp = nc.dram_tensor("bp", [H, DH], F32, kind="ExternalInput").ap()
    OUT = nc.dram_tensor("out", [BS, N, DOUT], F32, kind="ExternalOutput").ap()

    with tile.TileContext(nc) as tc:
        with (
            tc.tile_pool(name="singles", bufs=1) as singles,
            tc.tile_pool(name="xa", bufs=1) as xa,
            tc.tile_pool(name="xppool", bufs=2) as xppool,    # xp (dbl buf)
            tc.tile_pool(name="b1pool", bufs=1) as b1pool,    # xt
            tc.tile_pool(name="attnp", bufs=4) as attnp,      # attnT per head
            tc.tile_pool(name="outt", bufs=4) as outt,        # relu out ring
            tc.tile_pool(name="atp", bufs=2) as atp,          # A^T (dbl buf)
            tc.tile_pool(name="xptp", bufs=2) as xptp,        # XpT (dbl buf)
            tc.tile_pool(name="psT", bufs=2, space="PSUM") as psT,        # 4
            tc.tile_pool(name="psProj", bufs=2, space="PSUM") as psProj,  # 2
            tc.tile_pool(name="psXh", bufs=2, space="PSUM") as psXh,      # 2
        ):
            # ---- one-time setup ----
            ident = singles.tile([128, 128], F32, name="ident")
            make_identity(nc, ident)
            identb = singles.tile([128, 128], BF16, name="identb")
            make_identity(nc, identb)

            # Wp: [H, DIN, DH] -> stage [128, t, h, k] f32 -> bf16
            wp_stage = xa.tile([128, DT, H, DH], F32, tag="anat",
                               name="wp_stage")
            wp_r = Wp.rearrange("h (t p) k -> p t h k", p=128)
            for t in range(DT):
                nc.gpsimd.dma_start(out=wp_stage[:, t, :, :], in_=wp_r[:, t, :, :])
            wp_sb = singles.tile([128, DT, H, DH], BF16, name="wp_sb")
            nc.vector.tensor_copy(wp_sb, wp_stage)

            # Cpad2[:, q, :]: block-diagonal pair matrix. Rows 0:64 hold
            # C[2q]^T in cols 0:64; rows 64:128 hold C[2q+1]^T in cols
            # 64:128; zeros elsewhere. Built via PE transpose (PSUM rows
            # 0:64), a DMA partition-duplicate, then parity placement.
            c_stage = xa.tile([DH, H, DH], F32, tag="xnat", name="c_stage")
            nc.gpsimd.dma_start(out=c_stage, in_=C.rearrange("h j k -> j h k"))
            ps_ct = psT.tile([128, N], F32, tag="T", name="ps_ct")
            for h in range(H):
                nc.tensor.transpose(ps_ct[:DH, h * DH:(h + 1) * DH],
                                    c_stage[:, h, :], ident[:DH, :DH])
            ct_sb = singles.tile([128, H, DH], BF16, name="ct_sb")
            nc.vector.tensor_copy(
                ct_sb[:DH],
                ps_ct[:DH, :H * DH].rearrange("p (h j) -> p h j", h=H))
            nc.gpsimd.dma_start(out=ct_sb[DH:], in_=ct_sb[:DH])
            cpad2 = singles.tile([128, PAIRS, 128], BF16, name="cpad2")
            nc.vector.memset(cpad2, 0.0)
            for h in range(H):
                u, q = h % 2, h // 2
                sl = slice(u * DH, (u + 1) * DH)
                nc.vector.tensor_copy(cpad2[sl, q, sl], ct_sb[sl, h, :])

            # Persistent zero-padded XcT tiles (batch parity double buffer).
            # Rows (h%2)*64..+64 of slot h get rewritten per batch; the other
            # half stays zero forever.
            xct_bufs = []
            for pb in range(2):
                t_ = singles.tile([128, H, N], BF16, name=f"xct{pb}")
                nc.vector.memset(t_, 0.0)
                xct_bufs.append(t_)

            bias_sb = None
            if with_bias:
                # bp broadcast to all partitions: [128, (h k)] f32 (Xp layout)
                bias_sb = singles.tile([128, H * DH], F32, name="bias_sb")
                bp_flat = bp.rearrange("h k -> (h k)")
                bcast = bass.AP(tensor=bp_flat.tensor, offset=bp_flat.offset,
                                ap=[[0, 128]] + list(bp_flat.ap))
                nc.gpsimd.dma_start(out=bias_sb, in_=bcast)

            # ---- slot-interleaved software pipeline ----
            # Engine queues execute in EMISSION order, and the scores/tanh
            # stream is ACT-gated (tanh 1.1us vs 0.43us of PE matmul per
            # chunk). So emission interleaves three streams at tanh-slot
            # granularity: scores(b) chunks, aggregate chunks of the
            # previous pair, and projection pieces of batch b+1.
            def proj_start(b):
                """Emit input DMAs for batch b and return its context."""
                x_nat = xa.tile([128, 4, DIN], F32, tag="xnat", name="x_nat")
                for j in range(4):
                    nc.sync.dma_start(out=x_nat[:, j, :],
                                      in_=X[b, j * 128:(j + 1) * 128, :])
                a_h = xa.tile([128, 4, N], F32, tag="anat", name="a_h")
                for j2 in range(4):
                    nc.sync.dma_start(out=a_h[:, j2, :],
                                      in_=A[b, j2 * 128:(j2 + 1) * 128, :])
                return dict(
                    b=b,
                    x_nat=x_nat,
                    a_h=a_h,
                    xt_sb=b1pool.tile([128, DT, N], BF16, tag="xt",
                                      name="xt_sb"),
                    xp_sb=xppool.tile([128, NCH, H * DH], BF16, tag="xp",
                                      name="xp_sb"),
                    xpt_sb=xptp.tile([128, PAIRS, N], BF16, tag="xpt",
                                     name="xpt_sb"),
                    xct_sb=xct_bufs[b % 2],
                    at_sb=atp.tile([128, NCH, N], BF16, tag="at",
                                   name="at_sb"),
                    attn={},
                )

            def proj_pieces(ctx, deep=False):
                """48 deferred-emission pieces computing batch b's projection.
                Order matters: AT-nh0 early (frees a_h), Xp after the slots
                where the previous xp_sb parity buffer is still being read by
                the deferred aggregate, AT-nh1 last (a_h reuse)."""
                b = ctx["b"]
                x_hi = _hi_bf16(ctx["x_nat"])
                xt_sb, xp_sb = ctx["xt_sb"], ctx["xp_sb"]
                xpt_sb, xct_sb, at_sb = (ctx["xpt_sb"], ctx["xct_sb"],
                                         ctx["at_sb"])
                a_h = ctx["a_h"]
                pieces = []

                def xt_piece(t2, nh, k=0):
                    def f():
                        pool, tag = ((psT, "T") if deep and k % 2 else
                                     (psProj, "proj"))
                        ps_xt = pool.tile([128, 1024], BF16, tag=tag,
                                          name="ps_xt")
                        for dt in range(2):
                            t = 2 * t2 + dt
                            for j2 in range(4):
                                nc.tensor.transpose(
                                    ps_xt[:, dt * 512 + j2 * 128:
                                          dt * 512 + (j2 + 1) * 128],
                                    x_hi[:, j2, t * 128:(t + 1) * 128],
                                    identb)
                        nc.vector.tensor_copy(
                            xt_sb[:, 2 * t2:2 * t2 + 2,
                                  nh * 512:(nh + 1) * 512],
                            ps_xt.rearrange("p (t n) -> p t n", t=2))
                    return f

                def x_dma2_piece():
                    def f():
                        for j2 in range(4):
                            j = 4 + j2
                            nc.sync.dma_start(out=ctx["x_nat"][:, j2, :],
                                              in_=X[b, j * 128:(j + 1) * 128, :])
                    return f

                def at_piece(nh, i2, k=0):
                    def f():
                        a_hi = _hi_bf16(a_h)
                        pool, tag = ((psT, "T") if deep and k % 2 else
                                     (psProj, "proj"))
                        ps_at = pool.tile([128, 1024], BF16, tag=tag,
                                          name="ps_at")
                        for di in range(2):
                            i = 2 * i2 + di
                            for j2 in range(4):
                                nc.tensor.transpose(
                                    ps_at[:, di * 512 + j2 * 128:
                                          di * 512 + (j2 + 1) * 128],
                                    a_hi[:, j2, i * 128:(i + 1) * 128],
                                    identb)
                        nc.vector.tensor_copy(
                            at_sb[:, 2 * i2:2 * i2 + 2,
                                  nh * 512:(nh + 1) * 512],
                            ps_at.rearrange("p (i n) -> p i n", i=2))
                    return f

                def a_dma2_piece():
                    def f():
                        for j2 in range(4):
                            j = 4 + j2
                            nc.sync.dma_start(out=a_h[:, j2, :],
                                              in_=A[b, j * 128:(j + 1) * 128, :])
                    return f

                def xp_piece(j):
                    def f():
                        pool, tag = ((psT, "T") if deep and j % 2 else
                                     (psProj, "proj"))
                        ps_xp = pool.tile([128, H * DH], F32, tag=tag,
                                          name="ps_xp")
                        for t in range(DT):
                            nc.tensor.matmul(
                                ps_xp, xt_sb[:, t, j * 128:(j + 1) * 128],
                                wp_sb[:, t, :, :],
                                start=(t == 0), stop=(t == DT - 1))
                        if with_bias:
                            nc.vector.tensor_add(xp_sb[:, j, :], ps_xp,
                                                 bias_sb)
                        else:
                            nc.vector.tensor_copy(xp_sb[:, j, :], ps_xp)
                    return f

                def xpt_piece(j):
                    def f():
                        pool, tag = ((psT, "T") if deep and j % 2 else
                                     (psProj, "proj"))
                        ps_xpt = pool.tile([128, 512], BF16, tag=tag,
                                          name="ps_xpt")
                        for q in range(PAIRS):
                            nc.tensor.transpose(
                                ps_xpt[:, q * 128:(q + 1) * 128],
                                xp_sb[:, j, q * 128:(q + 1) * 128], identb)
                        nc.vector.tensor_copy(
                            xpt_sb[:, :, j * 128:(j + 1) * 128],
                            ps_xpt.rearrange("p (q m) -> p q m", q=PAIRS))
                    return f

                def xct_piece(q, nh):
                    def f():
                        pool, tag = ((psT, "T") if deep and nh else
                                     (psProj, "proj"))
                        ps_xct = pool.tile([128, 512], F32, tag=tag,
                                          name="ps_xct")
                        nc.tensor.matmul(ps_xct, cpad2[:, q, :],
                                         xpt_sb[:, q, nh * 512:(nh + 1) * 512],
                                         start=True, stop=True)
                        sl = slice(nh * 512, (nh + 1) * 512)
                        if nh == 0:
                            nc.vector.tensor_copy(xct_sb[:DH, 2 * q, sl],
                                                  ps_xct[:DH])
                            nc.vector.tensor_copy(xct_sb[DH:, 2 * q + 1, sl],
                                                  ps_xct[DH:])
                        else:
                            nc.scalar.activation(xct_sb[:DH, 2 * q, sl],
                                                 ps_xct[:DH], AF.Copy)
                            nc.scalar.activation(xct_sb[DH:, 2 * q + 1, sl],
                                                 ps_xct[DH:], AF.Copy)
                    return f

                for t2 in range(DT // 2):
                    pieces.append(xt_piece(t2, 0, t2))
                pieces.append(x_dma2_piece())
                for t2 in range(DT // 2):
                    pieces.append(xt_piece(t2, 1, t2))
                for i2 in range(NCH // 2):
                    pieces.append(at_piece(0, i2, i2))
                pieces.append(a_dma2_piece())
                for j in range(NCH):
                    pieces.append(xp_piece(j))
                for j in range(NCH):
                    pieces.append(xpt_piece(j))
                for q in range(PAIRS):
                    for nh in range(2):
                        pieces.append(xct_piece(q, nh))
                for i2 in range(NCH // 2):
                    pieces.append(at_piece(1, i2, i2))
                return pieces

            def scores_chunk(ctx, q, u, i, at_u):
                h = 2 * q + u
                xpt_sb, xct_sb, at_sb = (ctx["xpt_sb"], ctx["xct_sb"],
                                         ctx["at_sb"])
                ps_t = psT.tile([128, N], F32, tag="T", name="ps_t")
                for nh in range(2):
                    nc.tensor.matmul(
                        ps_t[:, nh * 512:(nh + 1) * 512],
                        xpt_sb[:, q, i * 128:(i + 1) * 128],
                        xct_sb[:, h, nh * 512:(nh + 1) * 512],
                        start=True, stop=True)
                nc.scalar.activation(at_u[:, i, :], ps_t, AF.Tanh)
                # Mask in [128, 2048] pairs (fewer DVE/GPSIMD instructions).
                # GPSIMD (slow, ~4.2us/instr) takes the first chunk-pairs of
                # each head so its serial chain overlaps the remaining tanh
                # slots; DVE (2x mode) takes the rest.
                if i % 4 == 3:
                    nc.vector.tensor_mul(at_u[:, i - 3:i + 1, :],
                                         at_u[:, i - 3:i + 1, :],
                                         at_sb[:, i - 3:i + 1, :])

            def agg_half(ctx, q, j, u, holder):
                xp_sb = ctx["xp_sb"]
                attn_t = ctx["attn"][q]
                b = ctx["b"]
                if u == 0:
                    holder[0] = psXh.tile([128, 2 * DH], F32, tag="xh",
                                          name="ps_xh")
                ps_xh = holder[0]
                h = 2 * q + u
                for i in range(NCH):
                    nc.tensor.matmul(
                        ps_xh[:, u * DH:(u + 1) * DH],
                        attn_t[u][:, i, j * 128:(j + 1) * 128],
                        xp_sb[:, i, h * DH:(h + 1) * DH],
                        start=(i == 0), stop=(i == NCH - 1))
                if u == 1:
                    o = outt.tile([128, 2 * DH], F32, tag="ot", name="o")
                    nc.vector.tensor_scalar_max(o, ps_xh, 0.0)
                    nc.sync.dma_start(
                        out=OUT[b, j * 128:(j + 1) * 128,
                                q * 128:(q + 1) * 128],
                        in_=o)

            from collections import deque
            from functools import partial

            ctx_next = proj_start(0)
            for f in proj_pieces(ctx_next, deep=True):
                f()
            cur = ctx_next
            agg_fill = deque()
            for b in range(1, n_batch + 1):
                pieces = deque()
                if b < n_batch:
                    ctx_next = proj_start(b)
                    pieces = deque(proj_pieces(ctx_next))
                slot = 0
                for q in range(PAIRS):
                    for u in range(2):
                        at_u = attnp.tile([128, NCH, N], BF16, tag="attnT",
                                          name="attn_t")
                        cur["attn"].setdefault(q, []).append(at_u)
                        for i in range(NCH):
                            scores_chunk(cur, q, u, i, at_u)
                            if agg_fill:
                                agg_fill.popleft()()
                            # at batch start, drain the deferred pair-3
                            # aggregate twice as fast so its attn tiles are
                            # free before this batch's pair 1 allocates them
                            if slot < 8 and agg_fill:
                                agg_fill.popleft()()
                            if slot % 4 != 3 and pieces:
                                pieces.popleft()()
                            slot += 1
                    holders = [[None] for _ in range(NCH)]
                    agg_fill.extend(
                        partial(agg_half, cur, q, j, u2, holders[j])
                        for j in range(NCH) for u2 in (0, 1))
                while pieces:
                    pieces.popleft()()
                cur = ctx_next
            while agg_fill:
                agg_fill.popleft()()

    nc.compile()
    return nc


_CACHED = {}


def _get_nc(with_bias: bool):
    if with_bias not in _CACHED:
        _CACHED[with_bias] = _build(with_bias)
    return _CACHED[with_bias]


def kernel(X, A, Wp, bp, C):
    global LAST_EXEC_NS, LAST_TRACE_DIR
    X = np.ascontiguousarray(np.asarray(X, dtype=np.float32))
    A = np.ascontiguousarray(np.asarray(A, dtype=np.float32))
    Wp = np.ascontiguousarray(np.asarray(Wp, dtype=np.float32))
    bp = np.ascontiguousarray(np.asarray(bp, dtype=np.float32))
    C = np.ascontiguousarray(np.asarray(C, dtype=np.float32))

    with_bias = bool(np.any(bp))
    nc = _get_nc(with_bias)

    in_maps = []
    for c in range(NCORES):
        m = {
            "X": X[c * BS:(c + 1) * BS],
            "A": A[c * BS:(c + 1) * BS],
            "Wp": Wp,
            "C": C,
        }
        if with_bias:
            m["bp"] = bp
        in_maps.append(m)

    trace = bool(int(os.environ.get("KERNEL_TRACE", "0")))
    res = run_bass_kernel_spmd(nc, in_maps, core_ids=list(range(NCORES)),
                               trace=trace)
    LAST_EXEC_NS = res.exec_time_ns
    if res.instructions_and_trace is not None:
        LAST_TRACE_DIR = res.instructions_and_trace[1]
    out = np.concatenate([res.results[c]["out"] for c in range(NCORES)], axis=0)
    return out.astype(np.float32)
